# revision 42
# baseline (speedup 1.0000x reference)
"""Causal self-attention (B=4, T=2048, C=2048, H=16, hd=128) on 8 trn2 cores.

Sharding: core = b*2 + half. Each core handles batch b and 8 heads
(half*8 .. half*8+7): tensor-parallel over heads within a batch, data
parallel over batch. Each core computes a partial out-projection
(contribution of its 8 heads); host sums the two partials per batch.

All tensors live in SBUF in bf16 (host pre-casts inputs); no DRAM
scratch roundtrips. Matmuls run bf16 (same PE rate as f32r, half the
DMA/SBUF). Device kernel per core:
  P1v: v = x @ Wv^T for all 8 heads (natural [t, d] layout), PSUM ->
       SBUF bf16 via ACT-engine copies.
  P1qk: per head, qT/kT = W^T-chunks @ xT with RoPE fused on DVE
       (even/odd dims pre-permuted into partition halves via host
       weight permutation; sin term via the pre-shuffled-sin trick:
       qf = qt*cos + shuffle(qt*sin_pre)).
  P2: per (head, 512-wide q strip): transposed scores sT[k,q] on PE
      (only causal 128-key blocks, diagonal blocks width-restricted),
      exp on ACT -> bf16, upper-triangle of each diagonal block zeroed
      by affine_select on the Pool engine, attV + ones-column sums
      accumulated on PE, normalization via PE broadcast of 1/sum.
      Score matmuls run two blocks ahead of attV (software pipeline);
      each strip's normalize chain is emitted inside the next strip to
      keep the PE queue dense.
  P3: partial out-proj from SBUF-resident yT (bf16), fp32 out.
"""

import numpy as np

import concourse.bass as bass
import concourse.tile as tile
from concourse import bacc, bass2jax, mybir

F32 = mybir.dt.float32
BF16 = mybir.dt.bfloat16

B = 4
T = 2048
C = 2048
HD = 128
HL = 8          # local heads per core
NCC = 16        # contraction chunks of 128 over C
NTB = 16        # t blocks of 128
NQS = 4         # q strips of 512
SW = 512
N_CORES = 8

# stream_shuffle mask: swap partition halves 0-15 <-> 16-31 within each
# 32-partition block (the rope-pair partner swap for the _PERM layout)
SWAP_MASK = list(range(16, 32)) + list(range(16))


def build_program(reps=1):
    nc = bacc.Bacc(None, target_bir_lowering=False)

    xT = nc.declare_dram_parameter("xT", [NCC, 128, T], BF16, isOutput=False)
    wq = nc.declare_dram_parameter("wq", [HL, 128, C], BF16, isOutput=False)
    wk = nc.declare_dram_parameter("wk", [HL, 128, C], BF16, isOutput=False)
    wv = nc.declare_dram_parameter("wv", [2, 128, 16 * SW], BF16, isOutput=False)
    wp = nc.declare_dram_parameter("wp", [HL, 128, C], BF16, isOutput=False)
    cs = nc.declare_dram_parameter("cs", [128, T], BF16, isOutput=False)
    sp = nc.declare_dram_parameter("sp", [128, T], BF16, isOutput=False)
    ones_in = nc.declare_dram_parameter("ones_in", [128, 128], BF16, isOutput=False)
    out = nc.declare_dram_parameter("out", [T, C], F32, isOutput=True)

    with tile.TileContext(nc) as tc:
        for _ in range(reps):
            _emit_body(nc, tc, xT, wq, wk, wv, wp, cs, sp, ones_in, out)

    nc.compile()
    return nc


def _emit_body(nc, tc, xT, wq, wk, wv, wp, cs, sp, ones_in, out):
        with tc.tile_pool(name="const", bufs=1) as cpool:
            ones_col = cpool.tile([128, 1], BF16, name="ones_col", tag="oc")
            nc.sync.dma_start(out=ones_col[:], in_=ones_in[:, 0:1])
            ones_row = cpool.tile([1, 128], BF16, name="ones_row", tag="orow")
            nc.sync.dma_start(out=ones_row[:], in_=ones_in[0:1, :])
            # tiles created now, loads issued later (after x/w) so the DMA
            # queue serves the PE-critical data first
            cs_sb = cpool.tile([128, T], BF16, name="cs_sb", tag="cs")
            sp_sb = cpool.tile([128, T], BF16, name="sp_sb", tag="sp")
            # first q-projection weight lives in the const pool so the q/k
            # phase doesn't wait for SBUF space freed by the v phase
            wsb0 = cpool.tile([128, C], BF16, name="wsb0", tag="w0")

            with tc.tile_pool(name="vres", bufs=1) as vpool:
                # v for all 8 heads, natural layout: [t_in_block, tb, h*128+d]
                v_sb = vpool.tile([128, NTB, HL * 128], BF16, name="v_sb", tag="v")

                with tc.tile_pool(name="qkres", bufs=1) as qkpool:
                    q_sb = [
                        qkpool.tile([128, T], BF16, name=f"q{h}", tag=f"q{h}")
                        for h in range(HL)
                    ]
                    k_sb = [
                        qkpool.tile([128, T], BF16, name=f"k{h}", tag=f"k{h}")
                        for h in range(HL)
                    ]

                    # ---------------- P1: projections ----------------
                    with tc.tile_pool(name="xin", bufs=1) as xpool:
                        xsb = [
                            xpool.tile([128, T], BF16, name=f"xsb{cc}", tag=f"x{cc}")
                            for cc in range(NCC)
                        ]

                        # phase A: v projection, chunk-major accumulation over
                        # 8 t-blocks at a time, paced with the x chunk DMAs
                        with tc.tile_pool(name="wvp", bufs=1) as wvpool:
                            wvsb = wvpool.tile(
                                [128, 2 * 16 * SW], BF16, name="wvsb", tag="wv"
                            )
                            nc.sync.dma_start(
                                out=wvsb[:, 0:SW], in_=wv[0][:, 0:SW]
                            )
                            nc.sync.dma_start(out=xsb[0][:], in_=xT[0])
                            nc.sync.dma_start(
                                out=wvsb[:, SW : 4 * SW], in_=wv[0][:, SW : 4 * SW]
                            )
                            nc.sync.dma_start(out=xsb[1][:], in_=xT[1])
                            nc.sync.dma_start(out=xsb[2][:], in_=xT[2])
                            nc.sync.dma_start(
                                out=wvsb[:, 4 * SW : 16 * SW],
                                in_=wv[0][:, 4 * SW : 16 * SW],
                            )
                            for cc in range(3, NCC):
                                nc.sync.dma_start(out=xsb[cc][:], in_=xT[cc])
                            nc.sync.dma_start(
                                out=wvsb[:, 16 * SW : 32 * SW], in_=wv[1]
                            )
                            nc.sync.dma_start(out=cs_sb[:], in_=cs[:])
                            nc.sync.dma_start(out=sp_sb[:], in_=sp[:])
                            nc.sync.dma_start(out=wsb0[:], in_=wq[0])

                            with tc.tile_pool(
                                name="vps", bufs=8, space="PSUM"
                            ) as vpspool:
                                for qd in range(2):
                                    for tbb in range(2):
                                        pvs = [
                                            vpspool.tile(
                                                [128, SW], F32, name=f"pv{i}",
                                                tag="vps",
                                            )
                                            for i in range(8)
                                        ]
                                        for cc in range(NCC):
                                            for i in range(8):
                                                tb = tbb * 8 + i
                                                nc.tensor.matmul(
                                                    pvs[i][:],
                                                    xsb[cc][
                                                        :, tb * 128 : (tb + 1) * 128
                                                    ],
                                                    wvsb[
                                                        :,
                                                        (16 * qd + cc) * SW
                                                        : (16 * qd + cc + 1) * SW,
                                                    ],
                                                    start=(cc == 0),
                                                    stop=(cc == NCC - 1),
                                                )
                                        for i in range(8):
                                            tb = tbb * 8 + i
                                            dst_ap = v_sb[
                                                :, tb, qd * SW : (qd + 1) * SW
                                            ]
                                            # alternate engines so the PSUM
                                            # banks drain twice as fast
                                            if i % 2 == 0:
                                                nc.scalar.copy(
                                                    out=dst_ap, in_=pvs[i][:]
                                                )
                                            else:
                                                nc.vector.tensor_copy(
                                                    dst_ap, pvs[i][:]
                                                )

                        # phase B: q/k projections + fused rope
                        with (
                            tc.tile_pool(name="wqk", bufs=2) as wpool,
                            tc.tile_pool(name="qkps", bufs=8, space="PSUM") as qkps,
                            tc.tile_pool(name="rope", bufs=1) as rpool,
                        ):
                            pairs = [(wq, q_sb, h) for h in range(HL)] + [
                                (wk, k_sb, h) for h in range(HL)
                            ]
                            for w_in, dst, h in pairs:
                                if w_in is wq and h == 0:
                                    wsb = wsb0  # preloaded in const pool
                                else:
                                    wsb = wpool.tile(
                                        [128, C], BF16, name=f"wsb_{h}", tag="wqk"
                                    )
                                    nc.sync.dma_start(out=wsb[:], in_=w_in[h])
                                pss = [
                                    qkps.tile(
                                        [128, SW], F32, name=f"pqk{s}", tag="qkps"
                                    )
                                    for s in range(NQS)
                                ]
                                for cc in range(NCC):
                                    for s in range(NQS):
                                        nc.tensor.matmul(
                                            pss[s][:],
                                            wsb[:, cc * 128 : (cc + 1) * 128],
                                            xsb[cc][:, s * SW : (s + 1) * SW],
                                            start=(cc == 0),
                                            stop=(cc == NCC - 1),
                                        )
                                for s in range(NQS):
                                    sl = slice(s * SW, (s + 1) * SW)
                                    qt = rpool.tile(
                                        [128, SW], BF16, name="qt", tag="qt", bufs=3
                                    )
                                    nc.scalar.copy(out=qt[:], in_=pss[s][:])
                                    u = rpool.tile(
                                        [128, SW], BF16, name="u", tag="u", bufs=2
                                    )
                                    nc.vector.tensor_mul(u[:], qt[:], sp_sb[:, sl])
                                    t1 = rpool.tile(
                                        [128, SW], BF16, name="t1", tag="t1", bufs=2
                                    )
                                    nc.vector.tensor_mul(t1[:], qt[:], cs_sb[:, sl])
                                    us = rpool.tile(
                                        [128, SW], BF16, name="us", tag="us", bufs=2
                                    )
                                    nc.vector.stream_shuffle(us[:], u[:], SWAP_MASK)
                                    nc.vector.tensor_add(dst[h][:, sl], t1[:], us[:])

                    # ---------------- P2: attention ----------------
                    with (
                        tc.tile_pool(name="yres", bufs=1) as ypool,
                        tc.tile_pool(name="wpp", bufs=1) as wppool,
                    ):
                        y_sb = [
                            ypool.tile([128, T], BF16, name=f"y{h}", tag=f"y{h}")
                            for h in range(HL)
                        ]
                        # prefetch out-proj weights during attention
                        wpsb = []
                        for cb in range(HL):
                            wt = wppool.tile(
                                [128, C], BF16, name=f"wp{cb}", tag=f"wp{cb}"
                            )
                            nc.sync.dma_start(out=wt[:], in_=wp[cb])
                            wpsb.append(wt)
                        with (
                            tc.tile_pool(name="stp", bufs=4, space="PSUM") as stpool,
                            tc.tile_pool(name="op", bufs=3, space="PSUM") as oppool,
                            tc.tile_pool(name="sump", bufs=1, space="PSUM") as sumpool,
                            tc.tile_pool(name="esb", bufs=6) as epool,
                            tc.tile_pool(name="nrm", bufs=2) as npool,
                        ):
                            # two sums rows in ONE PSUM bank; strips alternate
                            # rows so the next strip's sums never wait for the
                            # previous reciprocal read
                            sumbig = sumpool.tile(
                                [64, SW], F32, name="sumbig", tag="ps"
                            )
                            pending = None  # deferred normalize of previous strip

                            def normalize(po, psum, h, g):
                                recip = npool.tile(
                                    [1, SW], F32, name="recip", tag="recip"
                                )
                                nc.vector.reciprocal(recip[:], psum)
                                bsb = npool.tile([128, SW], F32, name="bsb", tag="bsb")
                                nc.gpsimd.partition_broadcast(bsb[:], recip[:])
                                nc.vector.tensor_mul(
                                    y_sb[h][:, g * SW : (g + 1) * SW], po[:], bsb[:]
                                )

                            for h in range(HL):
                                for g in range(NQS):
                                    nkb = 4 * g + 4
                                    par = 32 * ((h * NQS + g) % 2)
                                    po = oppool.tile([128, SW], F32, name="po", tag="po")
                                    psum = sumbig[par : par + 1, 0:SW]

                                    psts = []
                                    for kb in range(min(3, nkb)):
                                        off = 128 * max(0, kb - 4 * g)
                                        pst = stpool.tile(
                                            [128, SW], F32, name="pst", tag="pst"
                                        )
                                        nc.tensor.matmul(
                                            pst[:, off:SW],
                                            k_sb[h][:, kb * 128 : (kb + 1) * 128],
                                            q_sb[h][:, g * SW + off : (g + 1) * SW],
                                        )
                                        psts.append(pst)

                                    if pending is not None:
                                        normalize(*pending)
                                        pending = None

                                    for kb in range(nkb):
                                        off = 128 * max(0, kb - 4 * g)
                                        pst = psts[kb]
                                        esb = epool.tile(
                                            [128, SW], BF16, name="esb", tag="esb"
                                        )
                                        nc.scalar.activation(
                                            esb[:, off:SW],
                                            pst[:, off:SW],
                                            mybir.ActivationFunctionType.Exp,
                                        )
                                        if kb >= 4 * g:
                                            # zero e where k > q in the diagonal
                                            # 128x128 block
                                            nc.gpsimd.affine_select(
                                                out=esb[:, off : off + 128],
                                                in_=esb[:, off : off + 128],
                                                compare_op=mybir.AluOpType.is_ge,
                                                fill=0.0,
                                                base=0,
                                                pattern=[[1, 128]],
                                                channel_multiplier=-1,
                                            )
                                        kb2 = kb + 3
                                        if kb2 < nkb:
                                            off2 = 128 * max(0, kb2 - 4 * g)
                                            pst2 = stpool.tile(
                                                [128, SW], F32, name="pst", tag="pst"
                                            )
                                            nc.tensor.matmul(
                                                pst2[:, off2:SW],
                                                k_sb[h][:, kb2 * 128 : (kb2 + 1) * 128],
                                                q_sb[h][
                                                    :, g * SW + off2 : (g + 1) * SW
                                                ],
                                            )
                                            psts.append(pst2)
                                        nc.tensor.matmul(
                                            po[:, off:SW],
                                            v_sb[:, kb, HD * h : HD * (h + 1)],
                                            esb[:, off:SW],
                                            start=(kb == 0),
                                            stop=(kb == nkb - 1),
                                        )
                                        nc.tensor.matmul(
                                            sumbig[par : par + 1, off:SW],
                                            ones_col[:],
                                            esb[:, off:SW],
                                            start=(kb == 0),
                                            stop=(kb == nkb - 1),
                                        )
                                    pending = (po, psum, h, g)
                            normalize(*pending)

                        # ---------------- P3: out projection ----------------
                        with (
                            tc.tile_pool(name="fps", bufs=4, space="PSUM") as fpool,
                            tc.tile_pool(name="osb", bufs=3) as ospool,
                        ):
                            for tb in range(NTB):
                                for csi in range(4):
                                    pf = fpool.tile([128, SW], F32, name="pf", tag="pf")
                                    for cb in range(HL):
                                        nc.tensor.matmul(
                                            pf[:],
                                            y_sb[cb][:, tb * 128 : (tb + 1) * 128],
                                            wpsb[cb][:, csi * SW : (csi + 1) * SW],
                                            start=(cb == 0),
                                            stop=(cb == HL - 1),
                                        )
                                    osb = ospool.tile(
                                        [128, SW], F32, name="osb", tag="osb"
                                    )
                                    nc.vector.tensor_copy(osb[:], pf[:])
                                    nc.sync.dma_start(
                                        out=out[
                                            tb * 128 : (tb + 1) * 128,
                                            csi * SW : (csi + 1) * SW,
                                        ],
                                        in_=osb[:],
                                    )


# Per-head permutation of the 128 head dims: quadrant qd (32 partitions)
# holds rope pairs p = qd*16..qd*16+15 — even dims (2p) in slots 0..15,
# odd dims (2p+1) in slots 16..31. The rope partner swap is then a
# within-quadrant stream_shuffle by +-16.
_PERM = np.concatenate(
    [
        np.concatenate([2 * (qd * 16 + np.arange(16)) + r for r in (0, 1)])
        for qd in range(4)
    ]
)
# pair index held by each partition slot
_PAIR_OF_SLOT = np.concatenate(
    [np.tile(qd * 16 + np.arange(16), 2) for qd in range(4)]
)
# sign of the sin term on the PARTNER slot: +1 on even slots, -1 on odd
# (pre-shuffled-sin trick: qf = qt*cos + shuffle(qt*sp), where
# sp[p] = sign(partner(p)) * sin(freq_pair(p)) = -ss_signed[p])
_SP_SIGN = np.concatenate([np.repeat([1.0, -1.0], 16) for _ in range(4)])

_BF16 = mybir.dt.np(BF16)


def prepare_core_inputs(x, Wq, Wk, Wv, Wp):
    """Returns list of 8 input dicts, core = b*2 + half."""
    scale = 1.0 / np.sqrt(HD)

    inv_freq = (1.0 / (10000.0 ** (np.arange(0, HD, 2) / HD))).astype(np.float64)
    freqs = np.outer(inv_freq[_PAIR_OF_SLOT], np.arange(T, dtype=np.float64))
    cs = np.cos(freqs).astype(_BF16)
    sp = (np.sin(freqs) * _SP_SIGN[:, None]).astype(_BF16)

    halves = []
    for half in range(2):
        r0 = half * HL * HD  # first global row of this half's heads
        wq_in = np.empty((HL, 128, C), _BF16)
        wk_in = np.empty((HL, 128, C), _BF16)
        for h in range(HL):
            for arr, W, sc in ((wq_in, Wq, scale), (wk_in, Wk, 1.0)):
                Wh = W[r0 + h * HD : r0 + (h + 1) * HD][_PERM] * sc  # [128 d, C]
                # arr[h, p, cc*128+d] = Wh[d, cc*128+p]
                arr[h] = np.ascontiguousarray(
                    Wh.reshape(128, NCC, 128).transpose(2, 1, 0).reshape(128, C)
                ).astype(_BF16)
        Wv_half = Wv[r0 : r0 + HL * HD]  # [1024, C]
        wv_in = np.empty((2, 128, 16 * SW), _BF16)
        for qd in range(2):
            Wv4 = Wv_half[qd * SW : (qd + 1) * SW]  # [512 d4, C]
            wv_in[qd] = (
                Wv4.reshape(SW, NCC, 128).transpose(2, 1, 0).reshape(128, NCC * SW)
            ).astype(_BF16)
        wp_in = np.ascontiguousarray(
            Wp.T[r0 : r0 + HL * HD].reshape(HL, 128, C)
        ).astype(_BF16)
        halves.append((wq_in, wk_in, wv_in, wp_in))

    in_maps = []
    for b in range(B):
        xTb = (
            np.ascontiguousarray(x[b].T).reshape(NCC, 128, T).astype(_BF16)
        )
        for half in range(2):
            wq_in, wk_in, wv_in, wp_in = halves[half]
            in_maps.append(
                {
                    "xT": xTb,
                    "wq": wq_in,
                    "wk": wk_in,
                    "wv": wv_in,
                    "wp": wp_in,
                    "cs": cs,
                    "sp": sp,
                    "ones_in": np.ones((128, 128), _BF16),
                }
            )
    return in_maps


_RUNNER_CACHE = None


class _Runner:
    def __init__(self, sharded, mesh, in_names, out_names, out_avals, zero_shapes):
        self.sharded = sharded
        self.mesh = mesh
        self.in_names = in_names
        self.out_names = out_names
        self.out_avals = out_avals
        self.zero_shapes = zero_shapes
        self.body = None       # set by _make_runner
        self.n_params = None
        self.donate = None

    def make_chained(self, n):
        """jit that runs the kernel n times back-to-back per dispatch,
        serialized by threading outputs into the next call's donated
        output operands."""
        import jax
        from jax.experimental.shard_map import shard_map
        from jax.sharding import PartitionSpec

        body, n_params = self.body, self.n_params

        def chained(*args):
            ins = args[:n_params]
            outs = tuple(args[n_params:])
            for _ in range(n):
                outs = body(*ins, *outs)
            return outs

        n_outs = len(self.out_names)
        in_specs = (PartitionSpec("core"),) * (n_params + n_outs)
        out_specs = (PartitionSpec("core"),) * n_outs
        return jax.jit(
            shard_map(
                chained,
                mesh=self.mesh,
                in_specs=in_specs,
                out_specs=out_specs,
                check_rep=False,
            ),
            donate_argnums=self.donate,
            keep_unused=True,
        )

    def concat_inputs(self, in_maps):
        return [
            np.concatenate([np.asarray(m[name]) for m in in_maps], axis=0)
            for name in self.in_names
        ]

    def make_zeros(self):
        return [np.zeros((N_CORES * s[0], *s[1:]), d) for (s, d) in self.zero_shapes]

    def run(self, in_maps):
        out_arrs = self.sharded(*self.concat_inputs(in_maps), *self.make_zeros())
        return [
            {
                name: np.asarray(out_arrs[i]).reshape(
                    N_CORES, *self.out_avals[i].shape
                )[c]
                for i, name in enumerate(self.out_names)
            }
            for c in range(N_CORES)
        ]


def _make_runner(nc=None):
    """Compile the Bass program once and return a _Runner that reuses the
    jitted executable across calls. Mirrors bass2jax.run_bass_via_pjrt's
    multi-core branch."""
    import jax
    from jax.experimental.shard_map import shard_map
    from jax.sharding import Mesh, PartitionSpec

    if nc is None:
        nc = build_program()
    bass2jax.install_neuronx_cc_hook()

    partition_name = nc.partition_id_tensor.name if nc.partition_id_tensor else None
    in_names, out_names, out_avals, zero_shapes = [], [], [], []
    for alloc in nc.m.functions[0].allocations:
        if not isinstance(alloc, mybir.MemoryLocationSet):
            continue
        name = alloc.memorylocations[0].name
        if alloc.kind == "ExternalInput":
            if name != partition_name:
                in_names.append(name)
        elif alloc.kind == "ExternalOutput":
            shape = tuple(alloc.tensor_shape)
            dtype = mybir.dt.np(alloc.dtype)
            out_names.append(name)
            out_avals.append(jax.core.ShapedArray(shape, dtype))
            zero_shapes.append((shape, dtype))
    n_params = len(in_names)
    n_outs = len(out_avals)
    all_in_names = list(in_names) + list(out_names)
    if partition_name is not None:
        all_in_names.append(partition_name)
    donate = tuple(range(n_params, n_params + n_outs))

    def _body(*args):
        operands = list(args)
        if partition_name is not None:
            operands.append(bass2jax.partition_id_tensor())
        outs = bass2jax._bass_exec_p.bind(
            *operands,
            out_avals=tuple(out_avals),
            in_names=tuple(all_in_names),
            out_names=tuple(out_names),
            lowering_input_output_aliases=(),
            sim_require_finite=True,
            sim_require_nnan=True,
            nc=nc,
        )
        return tuple(outs)

    devices = jax.devices()[:N_CORES]
    mesh = Mesh(np.asarray(devices), ("core",))
    in_specs = (PartitionSpec("core"),) * (n_params + n_outs)
    out_specs = (PartitionSpec("core"),) * n_outs
    sharded = jax.jit(
        shard_map(
            _body, mesh=mesh, in_specs=in_specs, out_specs=out_specs, check_rep=False
        ),
        donate_argnums=donate,
        keep_unused=True,
    )
    r = _Runner(sharded, mesh, in_names, out_names, out_avals, zero_shapes)
    r.body = _body
    r.n_params = n_params
    r.donate = donate
    return r


def get_runner():
    global _RUNNER_CACHE
    if _RUNNER_CACHE is None:
        _RUNNER_CACHE = _make_runner()
    return _RUNNER_CACHE


def kernel(x, Wq, Wk, Wv, Wp):
    runner = get_runner()
    in_maps = prepare_core_inputs(
        np.asarray(x), np.asarray(Wq), np.asarray(Wk), np.asarray(Wv), np.asarray(Wp)
    )
    res = runner.run(in_maps)
    out = np.empty((B, T, C), np.float32)
    for b in range(B):
        np.add(res[2 * b]["out"], res[2 * b + 1]["out"], out=out[b])
    return out
